# revision 25
# baseline (speedup 1.0000x reference)
"""Biaffine span classifier kernel for 8 Trainium2 NeuronCores.

Math (per batch b, label o):
    start = relu(x @ W_start + b_start); end = relu(x @ W_end + b_end)
    rotate both with tiled-halves sinusoidal tables
    span[o,x,y] = startR[x,:] @ weight[o] @ endR[y,:]^T
    span = span*pad[y] - (1-pad[y])*NEG - NEG*tril(x>y)

Sharding: core c = b*2 + half handles batch b and labels [half*8, half*8+8).

The kernel is HBM-bound (9.4 MB of bf16 output + 2.4 MB of inputs per
core against a ~340-360 GB/s per-core DMA ceiling), so the design keeps
the DMA rings saturated from the moment the inputs finish landing:
  * Only the 36 upper-triangular 128x128 blocks per label are computed
    and written (as row bands [128k,128k+128) x cols [128k,1024)); the
    host fills the mask-determined rest exactly.
  * The whole matmul datapath runs fp16 (x, W, selector, rotation
    tables, wo, tmp, startR/endR), accumulating fp32 in PSUM:
      - no input upcasts, FWL (fast weight load) enabled (fp32_HIGH
        disables it), weight-side input DMA bytes halved
    fp16 rounding is 2^-11 per stage; end-to-end per-element error
    ~1e-2 < 2e-2 tolerance.
  * PSUM tiles span TWO banks [128, 2, 512]: the paired row-group
    matmuls (labels 2g, 2g+1) write bank 0/1 of one tile and a SINGLE
    cast instruction moves both to SBUF -- halving cast instruction
    count. Casts (every output element passes one) split DVE/ACT.
  * All 8 labels of a row band chunk stage in one [128, 8, 512] bf16
    tile and leave in ONE 3D DMA ([128,8,n]) -- 12 output DMA
    instructions total, split between the sync HWDGE ring and the
    gpsimd SWDGE ring.
  * Inputs load in criticality order on the sync ring; dummy matmuls
    warm the PE HAM clock window during the load so h1 prep runs at
    2.4 GHz.

Schedule: h=1 prep (proj -> relu -> selector mm -> rotate -> tmp) runs
first and unlocks bands 4-7 (rows 512+, h1-only data); h=0 prep
interleaves with those band casts/writes and unlocks bands 0-3 well
before the ring drains the h1 bands.
"""

import numpy as np

B, S, I, H, O = 4, 1024, 768, 64, 16
NCORES = 8
OH = O // 2  # 8 labels per core
NEG = 1.0e12
KT = I // 128  # 6 k-tiles over the input dim

# band xb covers rows [128xb, 128xb+128) x cols [128xb, 1024), computed in
# chunks of <= 512 columns (PSUM bank limit).
BAND_CHUNKS = {
    0: [(0, 512), (512, 1024)],
    1: [(128, 512), (512, 1024)],
    2: [(256, 512), (512, 1024)],
    3: [(384, 512), (512, 1024)],
    4: [(512, 1024)],
    5: [(640, 1024)],
    6: [(768, 1024)],
    7: [(896, 1024)],
}

_STATE = {}


def _tables():
    """Host-precomputed constants (mimic reference fp32 ops)."""
    position = np.arange(S, dtype=np.float32)
    idx = np.arange(H // 2, dtype=np.float32)
    expo = (np.float32(-2.0) * idx) / np.float32(H)
    inv_freq = np.power(np.float32(10000.0), expo).astype(np.float32)
    ang = position[:, None] * inv_freq[None, :]          # [S, 32] f32
    cos_h = np.cos(ang).astype(np.float32).T             # [32, S]
    sin_h = np.sin(ang).astype(np.float32).T
    cos2 = np.tile(cos_h, (4, 1))                        # [128, S]
    sin2 = np.tile(sin_h, (4, 1))
    cs2 = np.ascontiguousarray(
        np.stack([cos2, sin2], axis=1).astype(np.float16))  # [128, 2, S]
    # selector lhsT [128, 512]: 4 column blocks of 128, each mapping the
    # stacked [start;end] projection rows to DUPLICATED outputs (rows 0-63
    # and 64-127 identical). msw: out[2m] = -in[2m+1]; out[2m+1] = in[2m].
    sel = np.zeros((2 * H, 4 * 2 * H), np.float16)
    for d in range(2):  # duplicate halves of the output
        mo = 64 * d
        for j in range(H):
            sel[j, 0 + mo + j] = 1.0               # start dup
            sel[H + j, 256 + mo + j] = 1.0         # end dup
        for m in range(H // 2):
            sel[2 * m + 1, 128 + mo + 2 * m] = -1.0      # start swap
            sel[2 * m, 128 + mo + 2 * m + 1] = 1.0
            sel[H + 2 * m + 1, 384 + mo + 2 * m] = -1.0  # end swap
            sel[H + 2 * m, 384 + mo + 2 * m + 1] = 1.0
    return cs2, sel


def _build():
    import concourse.bacc as bacc
    import concourse.bass as bass
    import concourse.mybir as mybir
    from concourse import tile

    f32 = mybir.dt.float32
    f16 = mybir.dt.float16
    bf16 = mybir.dt.bfloat16
    AF = mybir.ActivationFunctionType
    ALU = mybir.AluOpType
    PSUM = bass.MemorySpace.PSUM

    nc = bacc.Bacc("TRN2", target_bir_lowering=False, debug=False,
                   num_devices=NCORES)

    # host-preswizzled [partition, ...] layouts: one large contiguous
    # descriptor run per partition
    xT_t = nc.dram_tensor("xTp", [128, KT, S], f16, kind="ExternalInput")
    wb_t = nc.dram_tensor("wbp", [128, KT, 2 * H], f16, kind="ExternalInput")
    b2_t = nc.dram_tensor("bias2", [2 * H, 1], f32, kind="ExternalInput")
    wo2_t = nc.dram_tensor("wo2", [2 * H, 2, 2 * H], f16,
                           kind="ExternalInput")
    cs_t = nc.dram_tensor("cs2", [2 * H, 2, S], f16, kind="ExternalInput")
    sel_t = nc.dram_tensor("sel4", [2 * H, 4 * 2 * H], f16,
                           kind="ExternalInput")
    out_t = nc.dram_tensor("out", [OH, S, S], bf16, kind="ExternalOutput")

    # [xb, p, o, y]: row = 128*xb + p; p leads o so the DMA iteration
    # order matches the SBUF stage layout [partition, label, col]
    out_b = out_t.ap().rearrange("o (xb p) y -> xb p o y", xb=8, p=128)

    xg = xT_t.ap()

    with tile.TileContext(nc) as tc:
        with tc.tile_pool(name="persist", bufs=1) as pp, \
             tc.tile_pool(name="scratch", bufs=3) as sp, \
             tc.tile_pool(name="stage", bufs=6) as stp, \
             tc.tile_pool(name="psu", bufs=1, space=PSUM) as psu:

            # per-half tiles so readers of one half never wait on
            # writers of the other
            xF1a = pp.tile([128, 3, 512], f16)
            xF1b = pp.tile([128, 3, 512], f16)
            xF0 = pp.tile([128, KT, 512], f16)
            wbT = pp.tile([128, KT, 2 * H], f16)
            sel4 = pp.tile([2 * H, 4 * 2 * H], f16)
            wo2 = pp.tile([2 * H, 2, 2 * H], f16)
            bias2 = pp.tile([2 * H, 1], f32)
            cs2h = [pp.tile([2 * H, 2, 512], f16, name=f"cs2h{h}")
                    for h in range(2)]
            startRh = [pp.tile([2 * H, 512], f16, name=f"startR{h}")
                       for h in range(2)]
            endRh = [pp.tile([2 * H, 512], f16, name=f"endR{h}")
                     for h in range(2)]
            # tmp split per half: h0 writers must not wait on h1 band
            # readers (tile deps are whole-tile)
            tmp2h = [pp.tile([2 * H, 4, 512], f16, name=f"tmp2h{h}")
                     for h in range(2)]
            wdum = pp.tile([128, 512], f16)

            sl1 = slice(512, 1024)
            sl0 = slice(0, 512)

            # one ring of 2-bank PSUM tiles [128, 2, 512] x 4 bufs = all
            # 8 banks; depth 4 lets the PE run ahead of the cast engines
            def ps2b():
                return psu.tile([128, 2, 512], f32, name="ps", tag="p",
                                bufs=4)

            # inputs on the sync HWDGE queue in critical-path order
            nc.sync.dma_start(wbT[:], wb_t.ap())
            nc.sync.dma_start(xF1a[:], xg[:, 0:3, sl1])
            nc.sync.dma_start(xF1b[:], xg[:, 3:6, sl1])
            nc.sync.dma_start(bias2[:], b2_t.ap())
            nc.sync.dma_start(sel4[:], sel_t.ap())
            nc.sync.dma_start(cs2h[1][:], cs_t.ap()[:, :, sl1])
            nc.sync.dma_start(wo2[:], wo2_t.ap())
            nc.sync.dma_start(xF0[:], xg[:, :, sl0])
            nc.sync.dma_start(cs2h[0][:], cs_t.ap()[:, :, sl0])

            # PE warm-up: dummy matmuls so the HAM window is at full
            # clock when the h1 projection starts (~3.4us ramp budget);
            # sized to end right as the projection inputs land
            nc.gpsimd.memset(wdum[:], 0.0)
            for i in range(7):
                ps_w = ps2b()
                nc.tensor.matmul(ps_w[:, i % 2, :], wdum[:, 0:128],
                                 wdum[:], start=True, stop=True)

            proj_ps = {}
            relu_of = {}

            def proj_h(h):
                ps2 = ps2b()
                if h == 1:
                    for kb in range(3):
                        nc.tensor.matmul(ps2[:, 0, :], wbT[:, kb, :],
                                         xF1a[:, kb, :],
                                         start=(kb == 0), stop=False)
                    for kb in range(3):
                        nc.tensor.matmul(ps2[:, 0, :], wbT[:, kb + 3, :],
                                         xF1b[:, kb, :],
                                         start=False, stop=(kb == 2))
                else:
                    for kb in range(KT):
                        nc.tensor.matmul(ps2[:, 0, :], wbT[:, kb, :],
                                         xF0[:, kb, :],
                                         start=(kb == 0),
                                         stop=(kb == KT - 1))
                proj_ps[h] = ps2

            def relu_h(h):
                relu2 = sp.tile([128, 512], f16, name="relu2")
                nc.scalar.activation(relu2[:], proj_ps[h][:, 0, :],
                                     AF.Relu, bias=bias2[:])
                relu_of[h] = relu2

            def rot_sel(h, side):
                relu2 = relu_of[h]
                q = 256 * side
                ps_r = ps2b()
                nc.tensor.matmul(ps_r[:, 0, :], sel4[:, q:q + 128],
                                 relu2[:], start=True, stop=True)
                nc.tensor.matmul(ps_r[:, 1, :], sel4[:, q + 128:q + 256],
                                 relu2[:], start=True, stop=True)
                return ps_r

            def rot_mul(h, ps_r, s0, s1, t_dw):
                # fused [cos;sin] DVE mul over columns [s0,s1); h0's runs
                # as 256-col pieces interleaved between band cast groups
                # so no single DMA-ring hole forms
                nc.vector.tensor_tensor(t_dw[:, :, s0:s1],
                                        ps_r[:, :, s0:s1],
                                        cs2h[h][:, :, s0:s1], ALU.mult)

            def rot_add(h, side, t_dw, eng):
                dst = (startRh if side == 0 else endRh)[h]
                eng.tensor_tensor(dst[:], t_dw[:, 0, :],
                                  t_dw[:, 1, :], ALU.add)

            def rot_h(h, side, add_eng):
                ps_r = rot_sel(h, side)
                t_dw = sp.tile([128, 2, 512], f16, name="t_dw")
                rot_mul(h, ps_r, 0, 512, t_dw)
                rot_add(h, side, t_dw, add_eng)

            def tmp_mm(pg, h):
                ps_t = ps2b()
                for ph in range(2):
                    nc.tensor.matmul(ps_t[:, ph, :],
                                     wo2[64 * ph:64 * ph + 64, pg, :],
                                     startRh[h][64 * ph:64 * ph + 64, :],
                                     start=True, stop=True,
                                     tile_position=(64 * ph, 0))
                return ps_t

            def tmp_cast(pg, h, ps_t):
                nc.scalar.copy(tmp2h[h][:, 2 * pg, :], ps_t[:, 0, :])
                nc.vector.tensor_copy(tmp2h[h][:, 2 * pg + 1, :],
                                      ps_t[:, 1, :])

            # output-cast engine per label pair (g): split DVE/ACT
            CAST_G = ["scalar", "vector", "scalar", "vector"]

            def cast(eng, dst, src):
                if eng == "vector":
                    nc.vector.tensor_copy(dst, src)
                else:
                    nc.scalar.copy(dst, src)

            def band(xb, dma_eng, cast_g=CAST_G):
                """All 4 label pairs of row band xb: paired matmuls into
                2-bank PSUM tiles, fused casts into a per-chunk 8-label
                stage tile, one 3D DMA per chunk. Per-chunk stage tiles
                keep chunk-1 casts off chunk-0's DMA dependency (tile
                deps are whole-tile)."""
                w0 = 128 * xb
                tm = tmp2h[xb // 4]
                t0 = w0 - 512 * (xb // 4)
                for (c0, c1) in BAND_CHUNKS[xb]:
                    n = c1 - c0
                    eh = endRh[1] if c0 >= 512 else endRh[0]
                    e0 = c0 - 512 if c0 >= 512 else c0
                    stAB = stp.tile([128, 8, 512], bf16, name="stAB")
                    for g in range(4):
                        ps_s = ps2b()
                        nc.tensor.matmul(ps_s[:, 0, 0:n],
                                         tm[0:64, g, t0:t0 + 128],
                                         eh[0:64, e0:e0 + n],
                                         start=True, stop=True,
                                         tile_position=(0, 0))
                        nc.tensor.matmul(ps_s[:, 1, 0:n],
                                         tm[64:128, g, t0:t0 + 128],
                                         eh[64:128, e0:e0 + n],
                                         start=True, stop=True,
                                         tile_position=(64, 0))
                        cast(cast_g[g], stAB[:, 2 * g:2 * g + 2, 0:n],
                             ps_s[:, :, 0:n])
                    dma_eng.dma_start(out_b[xb][:, :, c0:c1],
                                      stAB[:, :, 0:n])

            # h1 prep runs first (rot adds on DVE for latency) and
            # unlocks bands 7,6. h0 prep interleaves with the h1 band
            # stream: its selector/rotation DVE muls are shredded into
            # 256-col pieces between band cast groups, and its adds run
            # on gpsimd, so no single DMA-ring hole forms mid-stream.
            proj_h(1)
            relu_h(1)
            rot_h(1, 0, nc.vector)
            rot_h(1, 1, nc.vector)
            ps_t10 = tmp_mm(0, 1)
            ps_t11 = tmp_mm(1, 1)
            tmp_cast(0, 1, ps_t10)
            tmp_cast(1, 1, ps_t11)
            band(7, nc.sync)
            band(6, nc.gpsimd)
            proj_h(0)
            relu_h(0)
            ps_r0s = rot_sel(0, 0)
            t_dw0s = sp.tile([128, 2, 512], f16, name="t_dw")
            rot_mul(0, ps_r0s, 0, 256, t_dw0s)
            rot_mul(0, ps_r0s, 256, 512, t_dw0s)
            rot_add(0, 0, t_dw0s, nc.gpsimd)
            band(5, nc.sync)
            ps_r0e = rot_sel(0, 1)
            t_dw0e = sp.tile([128, 2, 512], f16, name="t_dw")
            rot_mul(0, ps_r0e, 0, 256, t_dw0e)
            rot_mul(0, ps_r0e, 256, 512, t_dw0e)
            rot_add(0, 1, t_dw0e, nc.gpsimd)
            band(4, nc.gpsimd)
            ps_t00 = tmp_mm(0, 0)
            ps_t01 = tmp_mm(1, 0)
            tmp_cast(0, 0, ps_t00)
            tmp_cast(1, 0, ps_t01)
            band(3, nc.gpsimd)
            band(2, nc.sync)
            band(1, nc.gpsimd)
            band(0, nc.sync)

    nc.compile()
    return nc


def _get_nc():
    if "nc" not in _STATE:
        _STATE["nc"] = _build()
    return _STATE["nc"]


def _make_in_maps(x, mask, W_start, b_start, W_end, b_end, weight):
    cs2, sel = _tables()
    x = np.asarray(x, np.float32)
    W_start = np.asarray(W_start, np.float32)
    W_end = np.asarray(W_end, np.float32)
    w_both = np.ascontiguousarray(np.concatenate([W_start, W_end], axis=1))
    bias2 = np.ascontiguousarray(
        np.concatenate([np.asarray(b_start, np.float32).reshape(H),
                        np.asarray(b_end, np.float32).reshape(H)]).reshape(
                            2 * H, 1))
    weight = np.asarray(weight, np.float32)
    # pre-swizzle to [partition, t, ...]; ship x and weights as fp16
    xTs = [np.ascontiguousarray(
        x[b].T.reshape(KT, 128, S).transpose(1, 0, 2).astype(np.float16))
        for b in range(B)]
    wbp = np.ascontiguousarray(
        w_both.reshape(KT, 128, 2 * H).transpose(1, 0, 2).astype(np.float16))
    wo2s = []
    for half in range(2):
        wl = weight[half * OH:(half + 1) * OH]
        wo2 = np.zeros((2 * H, 2, 2 * H), np.float16)
        for pg in range(2):
            for ph in range(2):
                for u in range(2):
                    o = 2 * (2 * pg + ph) + u
                    wo2[64 * ph:64 * ph + 64, pg, 64 * u:64 * u + 64] = \
                        wl[o].astype(np.float16)
        wo2s.append(np.ascontiguousarray(wo2))
    in_maps = []
    for c in range(NCORES):
        b, half = c // 2, c % 2
        in_maps.append({
            "xTp": xTs[b],
            "wbp": wbp,
            "bias2": bias2,
            "wo2": wo2s[half],
            "cs2": cs2,
            "sel4": sel,
        })
    return in_maps


def _assemble(outs, mask):
    """Gather per-core band outputs into the full fp32 result, filling the
    mask-determined entries (masked columns, below-diagonal region) with
    their exact fp32 values."""
    mask = np.asarray(mask, np.float32)
    full = np.empty((B, O, S, S), np.float32)
    for c in range(NCORES):
        b, half = c // 2, c % 2
        full[b, half * OH:(half + 1) * OH] = \
            np.asarray(outs[c]).astype(np.float32)
    tri = np.tri(S, S, -1, dtype=bool)  # [x, y]: x > y
    for b in range(B):
        pad = mask[b]
        cols0 = np.nonzero(pad == 0.0)[0]
        if cols0.size:
            full[b][:, :, cols0] = np.float32(-NEG)
        below = (np.float32(-NEG) * (np.float32(2.0) - pad)).astype(
            np.float32)                                   # [y]
        full[b][:, tri] = np.broadcast_to(below, (S, S))[tri]
    return full


def _execute(in_maps, trace=False):
    from concourse.bass_utils import run_bass_kernel_spmd
    nc = _get_nc()
    return run_bass_kernel_spmd(nc, in_maps, list(range(NCORES)), trace=trace)


def kernel(x, mask, W_start, b_start, W_end, b_end, weight):
    in_maps = _make_in_maps(x, mask, W_start, b_start, W_end, b_end, weight)
    res = _execute(in_maps)
    outs = [res.results[c]["out"] for c in range(NCORES)]
    return _assemble(outs, mask)


# revision 26
# speedup vs baseline: 82234.3443x; 82234.3443x over previous
"""Biaffine span classifier kernel for 8 Trainium2 NeuronCores.

Math (per batch b, label o):
    start = relu(x @ W_start + b_start); end = relu(x @ W_end + b_end)
    rotate both with tiled-halves sinusoidal tables
    span[o,x,y] = startR[x,:] @ weight[o] @ endR[y,:]^T
    span = span*pad[y] - (1-pad[y])*NEG - NEG*tril(x>y)

Sharding: core c = b*2 + half handles batch b and labels [half*8, half*8+8).

The kernel is HBM-bound (9.4 MB of bf16 output + 2.4 MB of inputs per
core against a ~340-360 GB/s per-core DMA ceiling), so the design keeps
the DMA rings saturated from the moment the inputs finish landing:
  * Only the 36 upper-triangular 128x128 blocks per label are computed
    and written (as row bands [128k,128k+128) x cols [128k,1024)); the
    host fills the mask-determined rest exactly.
  * The whole matmul datapath runs fp16 (x, W, selector, rotation
    tables, wo, tmp, startR/endR), accumulating fp32 in PSUM:
      - no input upcasts, FWL (fast weight load) enabled (fp32_HIGH
        disables it), weight-side input DMA bytes halved
    fp16 rounding is 2^-11 per stage; end-to-end per-element error
    ~1e-2 < 2e-2 tolerance.
  * PSUM tiles span TWO banks [128, 2, 512]: the paired row-group
    matmuls (labels 2g, 2g+1) write bank 0/1 of one tile and a SINGLE
    cast instruction moves both to SBUF -- halving cast instruction
    count. Casts (every output element passes one) split DVE/ACT.
  * All 8 labels of a row band chunk stage in one [128, 8, 512] bf16
    tile and leave in ONE 3D DMA ([128,8,n]) -- 12 output DMA
    instructions total, split between the sync HWDGE ring and the
    gpsimd SWDGE ring.
  * Inputs load in criticality order on the sync ring; dummy matmuls
    warm the PE HAM clock window during the load so h1 prep runs at
    2.4 GHz.

Schedule: h=1 prep (proj -> relu -> selector mm -> rotate -> tmp) runs
first and unlocks bands 4-7 (rows 512+, h1-only data); h=0 prep
interleaves with those band casts/writes and unlocks bands 0-3 well
before the ring drains the h1 bands.
"""

import numpy as np

B, S, I, H, O = 4, 1024, 768, 64, 16
NCORES = 8
OH = O // 2  # 8 labels per core
NEG = 1.0e12
KT = I // 128  # 6 k-tiles over the input dim

# band xb covers rows [128xb, 128xb+128) x cols [128xb, 1024), computed in
# chunks of <= 512 columns (PSUM bank limit).
BAND_CHUNKS = {
    0: [(0, 512), (512, 1024)],
    1: [(128, 512), (512, 1024)],
    2: [(256, 512), (512, 1024)],
    3: [(384, 512), (512, 1024)],
    4: [(512, 1024)],
    5: [(640, 1024)],
    6: [(768, 1024)],
    7: [(896, 1024)],
}

_STATE = {}


def _tables():
    """Host-precomputed constants (mimic reference fp32 ops)."""
    position = np.arange(S, dtype=np.float32)
    idx = np.arange(H // 2, dtype=np.float32)
    expo = (np.float32(-2.0) * idx) / np.float32(H)
    inv_freq = np.power(np.float32(10000.0), expo).astype(np.float32)
    ang = position[:, None] * inv_freq[None, :]          # [S, 32] f32
    cos_h = np.cos(ang).astype(np.float32).T             # [32, S]
    sin_h = np.sin(ang).astype(np.float32).T
    cos2 = np.tile(cos_h, (4, 1))                        # [128, S]
    sin2 = np.tile(sin_h, (4, 1))
    cs2 = np.ascontiguousarray(
        np.stack([cos2, sin2], axis=1).astype(np.float16))  # [128, 2, S]
    # selector lhsT [128, 512]: 4 column blocks of 128, each mapping the
    # stacked [start;end] projection rows to DUPLICATED outputs (rows 0-63
    # and 64-127 identical). msw: out[2m] = -in[2m+1]; out[2m+1] = in[2m].
    sel = np.zeros((2 * H, 4 * 2 * H), np.float16)
    for d in range(2):  # duplicate halves of the output
        mo = 64 * d
        for j in range(H):
            sel[j, 0 + mo + j] = 1.0               # start dup
            sel[H + j, 256 + mo + j] = 1.0         # end dup
        for m in range(H // 2):
            sel[2 * m + 1, 128 + mo + 2 * m] = -1.0      # start swap
            sel[2 * m, 128 + mo + 2 * m + 1] = 1.0
            sel[H + 2 * m + 1, 384 + mo + 2 * m] = -1.0  # end swap
            sel[H + 2 * m, 384 + mo + 2 * m + 1] = 1.0
    return cs2, sel


def _build():
    import concourse.bacc as bacc
    import concourse.bass as bass
    import concourse.mybir as mybir
    from concourse import tile

    f32 = mybir.dt.float32
    f16 = mybir.dt.float16
    bf16 = mybir.dt.bfloat16
    AF = mybir.ActivationFunctionType
    ALU = mybir.AluOpType
    PSUM = bass.MemorySpace.PSUM

    nc = bacc.Bacc("TRN2", target_bir_lowering=False, debug=False,
                   num_devices=NCORES)

    # host-preswizzled [partition, ...] layouts: one large contiguous
    # descriptor run per partition
    xT_t = nc.dram_tensor("xTp", [128, KT, S], f16, kind="ExternalInput")
    wb_t = nc.dram_tensor("wbp", [128, KT, 2 * H], f16, kind="ExternalInput")
    b2_t = nc.dram_tensor("bias2", [2 * H, 1], f32, kind="ExternalInput")
    wo2_t = nc.dram_tensor("wo2", [2 * H, 2, 2 * H], f16,
                           kind="ExternalInput")
    cs_t = nc.dram_tensor("cs2", [2 * H, 2, S], f16, kind="ExternalInput")
    sel_t = nc.dram_tensor("sel4", [2 * H, 4 * 2 * H], f16,
                           kind="ExternalInput")
    out_t = nc.dram_tensor("out", [OH, S, S], bf16, kind="ExternalOutput")

    # [xb, p, o, y]: row = 128*xb + p; p leads o so the DMA iteration
    # order matches the SBUF stage layout [partition, label, col]
    out_b = out_t.ap().rearrange("o (xb p) y -> xb p o y", xb=8, p=128)

    xg = xT_t.ap()

    with tile.TileContext(nc) as tc:
        with tc.tile_pool(name="persist", bufs=1) as pp, \
             tc.tile_pool(name="scratch", bufs=3) as sp, \
             tc.tile_pool(name="stage", bufs=6) as stp, \
             tc.tile_pool(name="psu", bufs=1, space=PSUM) as psu:

            # per-half tiles so readers of one half never wait on
            # writers of the other
            xF1a = pp.tile([128, 3, 512], f16)
            xF1b = pp.tile([128, 3, 512], f16)
            xF0 = pp.tile([128, KT, 512], f16)
            wbT = pp.tile([128, KT, 2 * H], f16)
            sel4 = pp.tile([2 * H, 4 * 2 * H], f16)
            wo2 = pp.tile([2 * H, 2, 2 * H], f16)
            bias2 = pp.tile([2 * H, 1], f32)
            cs2h = [pp.tile([2 * H, 2, 512], f16, name=f"cs2h{h}")
                    for h in range(2)]
            startRh = [pp.tile([2 * H, 512], f16, name=f"startR{h}")
                       for h in range(2)]
            endRh = [pp.tile([2 * H, 512], f16, name=f"endR{h}")
                     for h in range(2)]
            # tmp split per half: h0 writers must not wait on h1 band
            # readers (tile deps are whole-tile)
            tmp2h = [pp.tile([2 * H, 4, 512], f16, name=f"tmp2h{h}")
                     for h in range(2)]
            wdum = pp.tile([128, 512], f16)

            sl1 = slice(512, 1024)
            sl0 = slice(0, 512)

            # one ring of 2-bank PSUM tiles [128, 2, 512] x 4 bufs = all
            # 8 banks; depth 4 lets the PE run ahead of the cast engines
            def ps2b():
                return psu.tile([128, 2, 512], f32, name="ps", tag="p",
                                bufs=4)

            # inputs on the sync HWDGE queue in critical-path order
            nc.sync.dma_start(wbT[:], wb_t.ap())
            nc.sync.dma_start(xF1a[:], xg[:, 0:3, sl1])
            nc.sync.dma_start(xF1b[:], xg[:, 3:6, sl1])
            nc.sync.dma_start(bias2[:], b2_t.ap())
            nc.sync.dma_start(sel4[:], sel_t.ap())
            nc.sync.dma_start(cs2h[1][:], cs_t.ap()[:, :, sl1])
            nc.sync.dma_start(wo2[:], wo2_t.ap())
            nc.sync.dma_start(xF0[:], xg[:, :, sl0])
            nc.sync.dma_start(cs2h[0][:], cs_t.ap()[:, :, sl0])

            # PE warm-up: dummy matmuls so the HAM window is at full
            # clock when the h1 projection starts (~3.4us ramp budget);
            # sized to end right as the projection inputs land
            nc.gpsimd.memset(wdum[:], 0.0)
            for i in range(7):
                ps_w = ps2b()
                nc.tensor.matmul(ps_w[:, i % 2, :], wdum[:, 0:128],
                                 wdum[:], start=True, stop=True)

            proj_ps = {}
            relu_of = {}

            def proj_h(h):
                ps2 = ps2b()
                if h == 1:
                    for kb in range(3):
                        nc.tensor.matmul(ps2[:, 0, :], wbT[:, kb, :],
                                         xF1a[:, kb, :],
                                         start=(kb == 0), stop=False)
                    for kb in range(3):
                        nc.tensor.matmul(ps2[:, 0, :], wbT[:, kb + 3, :],
                                         xF1b[:, kb, :],
                                         start=False, stop=(kb == 2))
                else:
                    for kb in range(KT):
                        nc.tensor.matmul(ps2[:, 0, :], wbT[:, kb, :],
                                         xF0[:, kb, :],
                                         start=(kb == 0),
                                         stop=(kb == KT - 1))
                proj_ps[h] = ps2

            def relu_h(h):
                relu2 = sp.tile([128, 512], f16, name="relu2")
                nc.scalar.activation(relu2[:], proj_ps[h][:, 0, :],
                                     AF.Relu, bias=bias2[:])
                relu_of[h] = relu2

            def rot_sel(h, side):
                relu2 = relu_of[h]
                q = 256 * side
                ps_r = ps2b()
                nc.tensor.matmul(ps_r[:, 0, :], sel4[:, q:q + 128],
                                 relu2[:], start=True, stop=True)
                nc.tensor.matmul(ps_r[:, 1, :], sel4[:, q + 128:q + 256],
                                 relu2[:], start=True, stop=True)
                return ps_r

            def rot_mul(h, ps_r, s0, s1, t_dw):
                # fused [cos;sin] DVE mul over columns [s0,s1); h0's runs
                # as 256-col pieces interleaved between band cast groups
                # so no single DMA-ring hole forms
                nc.vector.tensor_tensor(t_dw[:, :, s0:s1],
                                        ps_r[:, :, s0:s1],
                                        cs2h[h][:, :, s0:s1], ALU.mult)

            def rot_add(h, side, t_dw, eng):
                dst = (startRh if side == 0 else endRh)[h]
                eng.tensor_tensor(dst[:], t_dw[:, 0, :],
                                  t_dw[:, 1, :], ALU.add)

            def rot_h(h, side, add_eng):
                ps_r = rot_sel(h, side)
                t_dw = sp.tile([128, 2, 512], f16, name="t_dw")
                rot_mul(h, ps_r, 0, 512, t_dw)
                rot_add(h, side, t_dw, add_eng)

            def tmp_mm(pg, h):
                ps_t = ps2b()
                for ph in range(2):
                    nc.tensor.matmul(ps_t[:, ph, :],
                                     wo2[64 * ph:64 * ph + 64, pg, :],
                                     startRh[h][64 * ph:64 * ph + 64, :],
                                     start=True, stop=True,
                                     tile_position=(64 * ph, 0))
                return ps_t

            def tmp_cast(pg, h, ps_t):
                nc.scalar.copy(tmp2h[h][:, 2 * pg, :], ps_t[:, 0, :])
                nc.vector.tensor_copy(tmp2h[h][:, 2 * pg + 1, :],
                                      ps_t[:, 1, :])

            # output-cast engine per label pair (g): split DVE/ACT
            CAST_G = ["scalar", "vector", "scalar", "vector"]

            def cast(eng, dst, src):
                if eng == "vector":
                    nc.vector.tensor_copy(dst, src)
                else:
                    nc.scalar.copy(dst, src)

            def band(xb, dma_eng, cast_g=CAST_G):
                """All 4 label pairs of row band xb: paired matmuls into
                2-bank PSUM tiles, fused casts into a per-chunk 8-label
                stage tile, one 3D DMA per chunk. Per-chunk stage tiles
                keep chunk-1 casts off chunk-0's DMA dependency (tile
                deps are whole-tile)."""
                w0 = 128 * xb
                tm = tmp2h[xb // 4]
                t0 = w0 - 512 * (xb // 4)
                for (c0, c1) in BAND_CHUNKS[xb]:
                    n = c1 - c0
                    eh = endRh[1] if c0 >= 512 else endRh[0]
                    e0 = c0 - 512 if c0 >= 512 else c0
                    stAB = stp.tile([128, 8, 512], bf16, name="stAB")
                    for g in range(4):
                        ps_s = ps2b()
                        nc.tensor.matmul(ps_s[:, 0, 0:n],
                                         tm[0:64, g, t0:t0 + 128],
                                         eh[0:64, e0:e0 + n],
                                         start=True, stop=True,
                                         tile_position=(0, 0))
                        nc.tensor.matmul(ps_s[:, 1, 0:n],
                                         tm[64:128, g, t0:t0 + 128],
                                         eh[64:128, e0:e0 + n],
                                         start=True, stop=True,
                                         tile_position=(64, 0))
                        cast(cast_g[g], stAB[:, 2 * g:2 * g + 2, 0:n],
                             ps_s[:, :, 0:n])
                    dma_eng.dma_start(out_b[xb][:, :, c0:c1],
                                      stAB[:, :, 0:n])

            # h1 prep runs first (rot adds on DVE for latency) and
            # unlocks ALL of bands 4-7; they run big-first so the DMA
            # ring builds deep backlog that absorbs the h0-prep DVE/ACT
            # intrusions (selector/rotation muls shredded into 256-col
            # pieces between band cast groups, adds on gpsimd).
            proj_h(1)
            relu_h(1)
            rot_h(1, 0, nc.vector)
            rot_h(1, 1, nc.vector)
            ps_t10 = tmp_mm(0, 1)
            ps_t11 = tmp_mm(1, 1)
            tmp_cast(0, 1, ps_t10)
            tmp_cast(1, 1, ps_t11)
            band(4, nc.sync)
            proj_h(0)
            relu_h(0)
            ps_r0s = rot_sel(0, 0)
            t_dw0s = sp.tile([128, 2, 512], f16, name="t_dw")
            rot_mul(0, ps_r0s, 0, 256, t_dw0s)
            rot_mul(0, ps_r0s, 256, 512, t_dw0s)
            rot_add(0, 0, t_dw0s, nc.gpsimd)
            band(5, nc.gpsimd)
            ps_r0e = rot_sel(0, 1)
            t_dw0e = sp.tile([128, 2, 512], f16, name="t_dw")
            rot_mul(0, ps_r0e, 0, 256, t_dw0e)
            rot_mul(0, ps_r0e, 256, 512, t_dw0e)
            rot_add(0, 1, t_dw0e, nc.gpsimd)
            band(6, nc.sync)
            ps_t00 = tmp_mm(0, 0)
            ps_t01 = tmp_mm(1, 0)
            tmp_cast(0, 0, ps_t00)
            tmp_cast(1, 0, ps_t01)
            band(7, nc.gpsimd)
            band(3, nc.gpsimd)
            band(2, nc.sync)
            band(1, nc.gpsimd)
            band(0, nc.sync, ["scalar", "vector", "scalar", "scalar"])

    nc.compile()
    return nc


def _get_nc():
    if "nc" not in _STATE:
        _STATE["nc"] = _build()
    return _STATE["nc"]


def _make_in_maps(x, mask, W_start, b_start, W_end, b_end, weight):
    cs2, sel = _tables()
    x = np.asarray(x, np.float32)
    W_start = np.asarray(W_start, np.float32)
    W_end = np.asarray(W_end, np.float32)
    w_both = np.ascontiguousarray(np.concatenate([W_start, W_end], axis=1))
    bias2 = np.ascontiguousarray(
        np.concatenate([np.asarray(b_start, np.float32).reshape(H),
                        np.asarray(b_end, np.float32).reshape(H)]).reshape(
                            2 * H, 1))
    weight = np.asarray(weight, np.float32)
    # pre-swizzle to [partition, t, ...]; ship x and weights as fp16
    xTs = [np.ascontiguousarray(
        x[b].T.reshape(KT, 128, S).transpose(1, 0, 2).astype(np.float16))
        for b in range(B)]
    wbp = np.ascontiguousarray(
        w_both.reshape(KT, 128, 2 * H).transpose(1, 0, 2).astype(np.float16))
    wo2s = []
    for half in range(2):
        wl = weight[half * OH:(half + 1) * OH]
        wo2 = np.zeros((2 * H, 2, 2 * H), np.float16)
        for pg in range(2):
            for ph in range(2):
                for u in range(2):
                    o = 2 * (2 * pg + ph) + u
                    wo2[64 * ph:64 * ph + 64, pg, 64 * u:64 * u + 64] = \
                        wl[o].astype(np.float16)
        wo2s.append(np.ascontiguousarray(wo2))
    in_maps = []
    for c in range(NCORES):
        b, half = c // 2, c % 2
        in_maps.append({
            "xTp": xTs[b],
            "wbp": wbp,
            "bias2": bias2,
            "wo2": wo2s[half],
            "cs2": cs2,
            "sel4": sel,
        })
    return in_maps


def _assemble(outs, mask):
    """Gather per-core band outputs into the full fp32 result, filling the
    mask-determined entries (masked columns, below-diagonal region) with
    their exact fp32 values."""
    mask = np.asarray(mask, np.float32)
    full = np.empty((B, O, S, S), np.float32)
    for c in range(NCORES):
        b, half = c // 2, c % 2
        full[b, half * OH:(half + 1) * OH] = \
            np.asarray(outs[c]).astype(np.float32)
    tri = np.tri(S, S, -1, dtype=bool)  # [x, y]: x > y
    for b in range(B):
        pad = mask[b]
        cols0 = np.nonzero(pad == 0.0)[0]
        if cols0.size:
            full[b][:, :, cols0] = np.float32(-NEG)
        below = (np.float32(-NEG) * (np.float32(2.0) - pad)).astype(
            np.float32)                                   # [y]
        full[b][:, tri] = np.broadcast_to(below, (S, S))[tri]
    return full


def _execute(in_maps, trace=False):
    from concourse.bass_utils import run_bass_kernel_spmd
    nc = _get_nc()
    return run_bass_kernel_spmd(nc, in_maps, list(range(NCORES)), trace=trace)


def kernel(x, mask, W_start, b_start, W_end, b_end, weight):
    in_maps = _make_in_maps(x, mask, W_start, b_start, W_end, b_end, weight)
    res = _execute(in_maps)
    outs = [res.results[c]["out"] for c in range(NCORES)]
    return _assemble(outs, mask)
